# revision 44
# baseline (speedup 1.0000x reference)
"""Multi-head attention (S=2048, B=2, E=1024, H=16) on 8 Trainium2 cores.

Sharding: data-parallel over batch (4 cores per batch element) x tensor-parallel
over heads (4 heads per core), Megatron-style: Wq/Wk/Wv column-sharded,
Wo row-sharded, per-core partial outputs summed (+bo) on host.

Design (v1 trace analysis showed ScalarE exp saturated at ~1.05us per
[128,1024] ACTIVATE = ~135us total; everything else is scheduled around
keeping that stream gap-free):

- qT/kT pair-packed [128, T]: partitions 0:64 = head 2p, 64:128 = head
  2p+1 (the projection PSUM layout -> single eviction per pair).
- Score matmuls row-packed via tile_position: the two K=64 head matmuls
  of a pair run concurrently in the PE array (row strips 0:64 / 64:128),
  writing the two 512-col halves (= 2 PSUM banks) of one score tile.
- Attention software-pipelined over blocks B[b] = (pair, sq) in sq-major
  order: scores+exp of block b interleave per-sk with PV of block b-1;
  normalize (sk4), o-proj (sk 7/9/11/13) of block b-2 fill the slack, so
  ScalarE runs exp back-to-back.
- Startup is DMA-critical-path ordered: wk, xk (k-proj runs k-outer
  across all 8 PSUM banks as tiles land), wq, xq[sq0] (first q-proj),
  wv, xv halves, deferred xq; v-proj and the other 6 q-proj blocks are
  interleaved into attention blocks 0-3 (ptr/ppo transient banks).
- PSUM: psc 2x[128,1024] scores + ppo 2x[65,512] PV accum (64 out rows
  + softmax-denominator row via the [v|1] ones-column trick) + ptr
  2x[128,512] transients = 8 banks.
- Normalization: denominator rows -> reciprocal -> K=1 broadcast matmul
  -> outT = outU * bcast(1/den), merged per block.
- Output written bf16 (f32 partials summed on host); drain o-proj spreads
  over 4 PSUM slots, DVE + ScalarE fin casts, and both HWDGE DMA queues.

Numerics: softmax without max-subtraction (scores ~ N(0,1), exp safe in
bf16 range); exp@[v|1] then divide == softmax@v; bq,bk folded into
projection evictions; bv added to v; bo added on host; matmuls bf16,
PSUM accumulation fp32. Measured vs fp32 reference: rel err ~7.1e-3.
"""

import numpy as np
import ml_dtypes
from contextlib import ExitStack

import concourse.bass as bass
import concourse.mybir as mybir
from concourse import bacc
import concourse.tile as tile
from concourse.bass_utils import run_bass_kernel_spmd

S, B, E, H, HD = 2048, 2, 1024, 16, 64
P = 128
NCORES = 8
CORES_PER_BATCH = 4
HEADS_PER_CORE = H // CORES_PER_BATCH      # 4
LOCAL_E = HEADS_PER_CORE * HD              # 256
VW = HD + 1                                # 65: [v_h | ones]
T = S                                      # tokens per core (one batch elem)
KT = E // P                                # 8 contraction tiles for projections
NPAIR = HEADS_PER_CORE // 2                # 2 head pairs
SQ_BLK = 512
NSQ = T // SQ_BLK                          # 4
NSK = T // P                               # 16
NBLK = NPAIR * NSQ                         # 8 attention blocks
F32 = mybir.dt.float32
F32R = mybir.dt.float32r
BF16 = mybir.dt.bfloat16
NPBF16 = ml_dtypes.bfloat16
EXPF = mybir.ActivationFunctionType.Exp


def _build_program():
    nc = bacc.Bacc("TRN2")

    xq = nc.dram_tensor("xq", [E, T], BF16, kind="ExternalInput")
    xk = nc.dram_tensor("xk", [E, T], BF16, kind="ExternalInput")
    xv = nc.dram_tensor("xv", [E, T], BF16, kind="ExternalInput")
    # weights pre-rearranged on host to [P, k, n] partition-major layouts
    wqt = nc.dram_tensor("wqt", [P, KT * LOCAL_E], BF16,
                         kind="ExternalInput")
    wkt = nc.dram_tensor("wkt", [P, KT * LOCAL_E], BF16,
                         kind="ExternalInput")
    wvt = nc.dram_tensor("wvt", [P, KT * LOCAL_E], BF16,
                         kind="ExternalInput")
    wot = nc.dram_tensor("wot", [P, (LOCAL_E // P) * E], BF16,
                         kind="ExternalInput")
    bqh = nc.dram_tensor("bqh", [LOCAL_E], F32, kind="ExternalInput")
    bkh = nc.dram_tensor("bkh", [LOCAL_E], F32, kind="ExternalInput")
    # per head: [bv_head (64), 1.0] -> 65 columns
    bvb = nc.dram_tensor("bvb", [HEADS_PER_CORE * VW], F32R,
                         kind="ExternalInput")
    onescr = nc.dram_tensor("onescr", [P], F32R, kind="ExternalInput")
    outp = nc.dram_tensor("outp", [E, T], BF16, kind="ExternalOutput")

    with ExitStack() as ctx:
        ctx.enter_context(nc.allow_low_precision(reason="bf16 matmul pipeline"))
        tc = ctx.enter_context(tile.TileContext(nc))
        xpool = ctx.enter_context(tc.tile_pool(name="xpool", bufs=8))
        wpool = ctx.enter_context(tc.tile_pool(name="wpool", bufs=1))
        qkpool = ctx.enter_context(tc.tile_pool(name="qkpool", bufs=4))
        vpool = ctx.enter_context(tc.tile_pool(name="vpool", bufs=1))
        opool = ctx.enter_context(tc.tile_pool(name="opool", bufs=2))
        epool = ctx.enter_context(tc.tile_pool(name="epool", bufs=18))
        fpool = ctx.enter_context(tc.tile_pool(name="fpool", bufs=4))
        spool = ctx.enter_context(tc.tile_pool(name="spool", bufs=4))
        cpool = ctx.enter_context(tc.tile_pool(name="cpool", bufs=1))
        # PSUM: psc 2x[128,1024] = 4 banks; po 2x[65,512] = 2 banks;
        #       ptr 2x[128,512] transient (bcast/o-proj/v-proj) = 2 banks.
        psc = ctx.enter_context(tc.tile_pool(name="psc", bufs=2, space="PSUM"))
        ppo = ctx.enter_context(tc.tile_pool(name="ppo", bufs=2, space="PSUM"))
        ptr = ctx.enter_context(tc.tile_pool(name="ptr", bufs=2, space="PSUM"))

        # ---- constants + weights (DMA order matters: wk then xk first) --
        onesr_sb = cpool.tile([1, P], F32R, tag="onesr")
        nc.sync.dma_start(onesr_sb[:], onescr[None, :])
        bvb_sb = cpool.tile([1, HEADS_PER_CORE * VW], F32R, tag="bvbrow")
        nc.sync.dma_start(bvb_sb[:], bvb[None, :])
        # per-pair bias columns [128, NPAIR]
        bq_sb = cpool.tile([P, NPAIR], F32, tag="bq")
        nc.sync.dma_start(bq_sb[:], bqh.rearrange("(m p) -> p m", p=P))
        bk_sb = cpool.tile([P, NPAIR], F32, tag="bk")
        nc.sync.dma_start(bk_sb[:], bkh.rearrange("(m p) -> p m", p=P))

        # DMA priority order: wk+xk (k-proj), wq+xq_n0 (first q-proj),
        # wv+xv (v-proj in blocks 0-1), xq_n1-3 (deferred q-proj), wo
        HT = T // 2
        wk_sb = wpool.tile([P, KT, LOCAL_E], BF16, tag="wk")
        nc.sync.dma_start(wk_sb.rearrange("p k n -> p (k n)"), wkt[:, :])
        xk_t = [xpool.tile([P, T], BF16, tag="xk", name=f"xk{k}")
                for k in range(KT)]
        for k in range(KT):
            nc.sync.dma_start(xk_t[k][:, 0:HT], xk[k * P:(k + 1) * P, 0:HT])
        wq_sb = wpool.tile([P, KT, LOCAL_E], BF16, tag="wq")
        nc.sync.dma_start(wq_sb.rearrange("p k n -> p (k n)"), wqt[:, :])
        xq_t = [xpool.tile([P, T], BF16, tag="xq", name=f"xq{k}")
                for k in range(KT)]
        for k in range(KT):
            nc.sync.dma_start(xq_t[k][:, 0:SQ_BLK],
                              xq[k * P:(k + 1) * P, 0:SQ_BLK])
        for k in range(KT):
            nc.sync.dma_start(xk_t[k][:, HT:T], xk[k * P:(k + 1) * P, HT:T])
        wv_sb = wpool.tile([P, KT, LOCAL_E], BF16, tag="wv")
        nc.sync.dma_start(wv_sb.rearrange("p k n -> p (k n)"), wvt[:, :])
        xv_t = [xpool.tile([P, T], BF16, tag="xv", name=f"xv{k}")
                for k in range(KT)]
        for half in range(2):
            for k in range(KT):
                nc.sync.dma_start(xv_t[k][:, half * HT:(half + 1) * HT],
                                  xv[k * P:(k + 1) * P,
                                     half * HT:(half + 1) * HT])
        for n in range(1, NSQ):
            for k in range(KT):
                nc.sync.dma_start(
                    xq_t[k][:, n * SQ_BLK:(n + 1) * SQ_BLK],
                    xq[k * P:(k + 1) * P, n * SQ_BLK:(n + 1) * SQ_BLK])
        wo_sb = wpool.tile([P, LOCAL_E // P, E], BF16, tag="wo")
        nc.sync.dma_start(wo_sb.rearrange("p k n -> p (k n)"), wot[:, :])

        # preload the exp table-set during the projection phase
        warm = spool.tile([1, 2], F32, tag="warm")
        nc.scalar.activation(warm[0:1, 0:2], bq_sb[0:1, 0:2], EXPF)

        # broadcast [bv_h | 1] over all 128 partitions via a K=1 outer product
        bvb_ps = ptr.tile([P, SQ_BLK], F32, tag="tr")
        nc.tensor.matmul(bvb_ps[:, 0:HEADS_PER_CORE * VW],
                         onesr_sb[0:1, :], bvb_sb[0:1, :],
                         start=True, stop=True)
        bvb_bc = cpool.tile([P, HEADS_PER_CORE * VW], F32, tag="bvbbc")
        nc.vector.tensor_copy(bvb_bc[:], bvb_ps[:, 0:HEADS_PER_CORE * VW])

        # ---- persistent activations -----------------------------------
        # pair-packed: partitions 0:64 = head 2m, 64:128 = head 2m+1
        qT = [qkpool.tile([P, T], BF16, tag="qk", name=f"qT{m}")
              for m in range(NPAIR)]
        kT = [qkpool.tile([P, T], BF16, tag="qk", name=f"kT{m}")
              for m in range(NPAIR)]
        # v buffer: per sk-tile, per head: [v_h (64 cols) | ones (1 col)]
        vbuf = vpool.tile([P, NSK, HEADS_PER_CORE * VW], BF16, tag="v")
        for tt in range(NSK):
            nc.vector.tensor_copy(
                vbuf.rearrange("p s (h c) -> p s h c", c=VW)
                [:, tt, :, HD:HD + 1],
                bvb_bc.rearrange("p (h c) -> p h c", c=VW)[:, :, HD:HD + 1])
        # normalized attention outputs per pair [128, T] (pair-packed rows)
        outT = [opool.tile([P, T], BF16, tag="oT", name=f"outT{m}")
                for m in range(NPAIR)]

        # ---- Q/K projections: pair-packed eviction --------------------
        def qk_proj_blk(xt, w_sb, bias_sb, dsts, n, m, pool):
            if pool is psc:
                ps = psc.tile([P, 2 * SQ_BLK], F32, tag="sc",
                              name=f"qkp{n}_{m}")
            else:
                ps = ptr.tile([P, SQ_BLK], F32, tag="tr",
                              name=f"qkp{n}_{m}")
            use_scalar = pool is psc
            for k in range(KT):
                nc.tensor.matmul(
                    ps[:, 0:SQ_BLK],
                    w_sb[:, k, m * P:(m + 1) * P],
                    xt[k][:, n * SQ_BLK:(n + 1) * SQ_BLK],
                    start=(k == 0), stop=(k == KT - 1))
            if use_scalar:
                nc.scalar.add(
                    dsts[m][:, n * SQ_BLK:(n + 1) * SQ_BLK],
                    ps[:, 0:SQ_BLK], bias_sb[:, m:m + 1])
            else:
                nc.vector.tensor_scalar_add(
                    dsts[m][:, n * SQ_BLK:(n + 1) * SQ_BLK],
                    ps[:, 0:SQ_BLK], bias_sb[:, m:m + 1])

        # upfront K-proj covers only token-half 0 (sq blocks 0,1) so the
        # first scores wait on 2MB of xk + 1MB of xq instead of all 4MB;
        # token-half 1 (pass B) and q-proj(m1) are deferred into block 0
        kpsA = psc.tile([P, 2 * SQ_BLK], F32, tag="sc", name="kpsA")
        kps1t = [ptr.tile([P, SQ_BLK], F32, tag="tr", name=f"kpA1_{n}")
                 for n in range(2)]
        for k in range(KT):
            for n in range(2):
                nc.tensor.matmul(
                    kpsA[:, n * SQ_BLK:(n + 1) * SQ_BLK],
                    wk_sb[:, k, 0:P],
                    xk_t[k][:, n * SQ_BLK:(n + 1) * SQ_BLK],
                    start=(k == 0), stop=(k == KT - 1))
            for n in range(2):
                nc.tensor.matmul(
                    kps1t[n][:, 0:SQ_BLK],
                    wk_sb[:, k, P:2 * P],
                    xk_t[k][:, n * SQ_BLK:(n + 1) * SQ_BLK],
                    start=(k == 0), stop=(k == KT - 1))
        for n in range(2):
            nc.scalar.add(kT[0][:, n * SQ_BLK:(n + 1) * SQ_BLK],
                          kpsA[:, n * SQ_BLK:(n + 1) * SQ_BLK],
                          bk_sb[:, 0:1])
        for n in range(2):
            nc.vector.tensor_scalar_add(
                kT[1][:, n * SQ_BLK:(n + 1) * SQ_BLK],
                kps1t[n][:, 0:SQ_BLK], bk_sb[:, 1:2])
        qk_proj_blk(xq_t, wq_sb, bq_sb, qT, 0, 0, psc)

        def kproj_passB(m):
            ts = [ptr.tile([P, SQ_BLK], F32, tag="tr", name=f"kpB{m}_{n}")
                  for n in range(2)]
            for k in range(KT):
                for n in range(2):
                    nc.tensor.matmul(
                        ts[n][:, 0:SQ_BLK],
                        wk_sb[:, k, m * P:(m + 1) * P],
                        xk_t[k][:, (2 + n) * SQ_BLK:(3 + n) * SQ_BLK],
                        start=(k == 0), stop=(k == KT - 1))
            for n in range(2):
                nc.vector.tensor_scalar_add(
                    kT[m][:, (2 + n) * SQ_BLK:(3 + n) * SQ_BLK],
                    ts[n][:, 0:SQ_BLK], bk_sb[:, m:m + 1])

        # ---- V projection: v[t, o] = (xT[k,t]).T @ WvT[k, o] (+ bv) ----
        # deferred into attention blocks 0-1 (xv DMA lands mid-block 0)
        def v_proj(tt):
            ps = ptr.tile([P, SQ_BLK], F32, tag="tr", name=f"vps{tt}")
            for k in range(KT):
                nc.tensor.matmul(
                    ps[:, 0:LOCAL_E],
                    xv_t[k][:, tt * P:(tt + 1) * P],
                    wv_sb[:, k, :],
                    start=(k == 0), stop=(k == KT - 1))
            nc.vector.tensor_tensor(
                vbuf.rearrange("p s (h c) -> p s h c", c=VW)[:, tt, :, 0:HD],
                ps[:, 0:LOCAL_E].rearrange("p (h c) -> p h c", c=HD),
                bvb_bc.rearrange("p (h c) -> p h c", c=VW)[:, :, 0:HD],
                mybir.AluOpType.add)

        # ---- software-pipelined attention -----------------------------
        # sq-major block order: output projection of sq can run (and its
        # DMA drain) two blocks after (pr1, sq) completes
        BLKS = [(m, sq) for sq in range(NSQ) for m in range(NPAIR)]
        # deferred q-projection blocks: (bi, sk) -> (n, m); deadline for
        # qT[m][:, n] is block 2n+m (sq-major)
        QPROJ_SCHED = {(1, 10): (1, 0), (1, 15): (1, 1),
                       (2, 2): (2, 0), (2, 15): (2, 1),
                       (4, 2): (3, 0), (4, 10): (3, 1)}

        exs = {}       # b -> list of 16 ex tiles
        po_t = {}      # b -> [po0, po1]
        outU_t = {}    # b -> [128, SQ_BLK] unnormalized (pair-packed rows)
        dn_t = {}      # b -> [2, SQ_BLK] denominators

        FE_A = 128.0 * 1.4426950408889634          # 2^7 * log2(e)
        FE_B = 127.0 * 128.0 - 128.0 * 0.0573 + 0.5  # bias + err-center + rnd

        def em_scores_exp(b, sk, dve=False):
            m, sq = BLKS[b]
            sqs = slice(sq * SQ_BLK, (sq + 1) * SQ_BLK)
            sks = slice(sk * P, (sk + 1) * P)
            ps = psc.tile([P, 2 * SQ_BLK], F32, tag="sc",
                          name=f"sc{b}_{sk}")
            # two K=64 head matmuls packed as PE row-tiles (0,0) and (64,0)
            nc.tensor.matmul(ps[:, 0:SQ_BLK],
                             kT[m][0:HD, sks], qT[m][0:HD, sqs],
                             start=True, stop=True)
            nc.tensor.matmul(ps[:, SQ_BLK:2 * SQ_BLK],
                             kT[m][HD:P, sks], qT[m][HD:P, sqs],
                             start=True, stop=True)
            ex = epool.tile([P, 2 * SQ_BLK], BF16, tag="e",
                            name=f"ex{b}_{sk}")
            if dve:
                # fast-exp bit trick on the (otherwise idle) vector engine:
                # int16(s*FE_A + FE_B) bitcast as bf16 ~ exp(s), +-4.3%
                nc.vector.tensor_scalar(
                    ex[:].bitcast(mybir.dt.int16), ps[:], FE_A, FE_B,
                    mybir.AluOpType.mult, mybir.AluOpType.add)
            else:
                nc.scalar.activation(ex[:], ps[:], EXPF)
            exs[b].append(ex)

        def em_pv(b, sk):
            m, sq = BLKS[b]
            ex = exs[b][sk]
            for X in range(2):
                h = 2 * m + X
                nc.tensor.matmul(
                    po_t[b][X][0:VW, :],
                    vbuf[:, sk, h * VW:(h + 1) * VW],
                    ex[:, X * SQ_BLK:(X + 1) * SQ_BLK],
                    start=(sk == 0), stop=(sk == NSK - 1))

        def em_evict_po(b, drain=False):
            # right after PV(b, 15): free the two po banks quickly; in the
            # drain the scalar engine is idle and takes the outU copies
            oU = spool.tile([P, SQ_BLK], BF16, tag="oU", bufs=2,
                            name=f"oU{b}")
            dn = [spool.tile([1, SQ_BLK], F32, tag="dn", bufs=2,
                             name=f"dn{b}_{X}") for X in range(2)]
            for X in range(2):
                if drain:
                    nc.scalar.copy(oU[X * HD:(X + 1) * HD, :],
                                   po_t[b][X][0:HD, :])
                else:
                    nc.vector.tensor_copy(oU[X * HD:(X + 1) * HD, :],
                                          po_t[b][X][0:HD, :])
                nc.vector.tensor_copy(dn[X][:], po_t[b][X][HD:VW, :])
            outU_t[b] = oU
            dn_t[b] = dn

        bc_t = {}

        def em_normalize_x(b, X, drain=False):
            if X == 0:
                bc_t[b] = spool.tile([P, SQ_BLK], BF16, tag="bc", bufs=2,
                                     name=f"bc{b}")
            rc = spool.tile([1, SQ_BLK], F32, tag="rc", bufs=2,
                            name=f"rc{b}_{X}")
            nc.vector.reciprocal_approx_fast(rc[:], dn_t[b][X][:])
            rcr = spool.tile([1, SQ_BLK], F32R, tag="rcr", bufs=2,
                             name=f"rcr{b}_{X}")
            nc.vector.tensor_copy(rcr[:], rc[:])
            pb = ptr.tile([P, SQ_BLK], F32, tag="tr",
                          name=f"pb{b}_{X}")
            nc.tensor.matmul(pb[0:HD, :], onesr_sb[0:1, 0:HD], rcr[:],
                             start=True, stop=True)
            if drain:
                nc.scalar.copy(bc_t[b][X * HD:(X + 1) * HD, :], pb[0:HD, :])
            else:
                nc.vector.tensor_copy(bc_t[b][X * HD:(X + 1) * HD, :],
                                      pb[0:HD, :])

        def em_normalize_fin(b):
            m, sq = BLKS[b]
            sqs = slice(sq * SQ_BLK, (sq + 1) * SQ_BLK)
            nc.vector.tensor_tensor(outT[m][:, sqs], outU_t[b][:],
                                    bc_t[b][:], mybir.AluOpType.mult)

        fd_t = {}

        def em_oproj(sq, m_range, drain=False):
            sqs = slice(sq * SQ_BLK, (sq + 1) * SQ_BLK)
            if m_range[0] == 0:
                # batch output into 2 wide DMAs per sq (8 chunk-descriptors
                # pay ~1.2us completion each on the queue)
                fd_t[sq] = [fpool.tile([P, 4, SQ_BLK], BF16, tag="fd",
                                       bufs=2, name=f"fd{sq}_{g}")
                            for g in range(2)]
            fd = fd_t[sq]
            ov = outp.rearrange("(g p) t -> p g t", p=P)
            for mm in m_range:
                # in the drain, spread across 4 PSUM slots and both the DVE
                # and the (now idle) scalar engine / scalar DMA queue
                if drain and mm % 2 == 1:
                    pt = ppo.tile([P, SQ_BLK], F32, tag="po",
                                  name=f"op{sq}_{mm}")
                else:
                    pt = ptr.tile([P, SQ_BLK], F32, tag="tr",
                                  name=f"op{sq}_{mm}")
                for kb in range(NPAIR):
                    nc.tensor.matmul(
                        pt[:],
                        wo_sb[:, kb, mm * P:(mm + 1) * P],
                        outT[kb][:, sqs],
                        start=(kb == 0), stop=(kb == NPAIR - 1))
                dst = fd[mm // 4][:, mm % 4, :]
                if drain and mm % 2 == 1:
                    nc.scalar.copy(dst, pt[:])
                else:
                    nc.vector.tensor_copy(dst, pt[:])
                if mm == 3:
                    nc.sync.dma_start(ov[:, 0:4, sqs], fd[0][:])
                if mm == 7:
                    if drain:
                        nc.scalar.dma_start(ov[:, 4:8, sqs], fd[1][:])
                    else:
                        nc.sync.dma_start(ov[:, 4:8, sqs], fd[1][:])

        opre = []
        for bi in range(NBLK + 1):
            drain = bi == NBLK
            if not drain:
                exs[bi] = []
                po_t[bi] = [ppo.tile([VW, SQ_BLK], F32, tag="po",
                                     name=f"po{bi}_{X}") for X in range(2)]
            for sk in range(NSK):
                if bi >= 1:
                    em_pv(bi - 1, sk)
                if not drain:
                    # note: offloading some tiles' exp to the DVE via the
                    # fast-exp bit trick (dve=True) is numerically fine
                    # (rel err ~9e-3) but measured net-neutral-to-worse:
                    # the DVE's bursty FIFO delivers the tiles late and the
                    # psc WAR then stalls the PE score stream.
                    em_scores_exp(bi, sk)
                # deferred startup work in block 0's slack: k-proj token
                # half 1, q-proj (sq0, m1), then the v-projection
                if bi == 0 and sk == 0:
                    kproj_passB(0)
                if bi == 0 and sk == 2:
                    qk_proj_blk(xq_t, wq_sb, bq_sb, qT, 0, 1, psc)
                if bi == 0 and sk == 4:
                    kproj_passB(1)
                if bi == 0 and sk >= 9:
                    v_proj(sk - 9)
                if bi == 1 and sk < 9:
                    v_proj(sk + 7)
                if (bi, sk) in QPROJ_SCHED:
                    n, m = QPROJ_SCHED[(bi, sk)]
                    qk_proj_blk(xq_t, wq_sb, bq_sb, qT, n, m, ptr)
                if drain and sk == 6:
                    for g in range(2):
                        opre.append(psc.tile([P, 2 * SQ_BLK], F32,
                                             tag="sc", name=f"opre{g}"))
                    sq3 = NSQ - 1
                    for mm in range(4):
                        nc.tensor.matmul(
                            opre[mm // 2][:, (mm % 2) * SQ_BLK:
                                          (mm % 2 + 1) * SQ_BLK],
                            wo_sb[:, 0, mm * P:(mm + 1) * P],
                            outT[0][:, sq3 * SQ_BLK:(sq3 + 1) * SQ_BLK],
                            start=True, stop=False)
                if bi >= 1 and sk == NSK - 1:
                    em_evict_po(bi - 1, drain=drain)
                if bi >= 2:
                    # stagger block bi-2's post-processing into this block
                    nb = bi - 2
                    if sk == 4:
                        em_normalize_x(nb, 0)
                        em_normalize_x(nb, 1)
                        em_normalize_fin(nb)
                    # o-proj of sq once both its pair-blocks are normalized
                    if bi >= 3 and bi % 2 == 1:
                        osq = (bi - 3) // 2
                        if sk in (7, 9, 11, 13):
                            i0 = (sk - 7) // 2 * 2
                            em_oproj(osq, range(i0, i0 + 2))
        # tail: the last block's normalize + o-proj for the last sq.
        # m0-3's kb=0 contribution was pre-accumulated during the drain
        # (opre tiles); finish those and run m4-7 in full.
        em_normalize_x(NBLK - 1, 0)
        em_normalize_x(NBLK - 1, 1)
        em_normalize_fin(NBLK - 1)
        sq3 = NSQ - 1
        sqs3 = slice(sq3 * SQ_BLK, (sq3 + 1) * SQ_BLK)
        fd3 = [fpool.tile([P, 4, SQ_BLK], BF16, tag="fd", bufs=2,
                          name=f"fd3_{g}") for g in range(2)]
        ov3 = outp.rearrange("(g p) t -> p g t", p=P)
        for mm in range(4):
            pt = opre[mm // 2][:, (mm % 2) * SQ_BLK:(mm % 2 + 1) * SQ_BLK]
            nc.tensor.matmul(pt, wo_sb[:, 1, mm * P:(mm + 1) * P],
                             outT[1][:, sqs3], start=False, stop=True)
            if mm % 2 == 1:
                nc.scalar.copy(fd3[0][:, mm - 1:mm + 1, :],
                               opre[mm // 2][:, 0:2 * SQ_BLK]
                               .rearrange("p (j c) -> p j c", c=SQ_BLK))
            if mm == 3:
                nc.sync.dma_start(ov3[:, 0:4, sqs3], fd3[0][:])
        for mm in range(4, 8):
            if mm < 6:
                pt = ppo.tile([P, SQ_BLK], F32, tag="po",
                              name=f"op3_{mm}")[:, 0:SQ_BLK]
            else:
                pt = ptr.tile([P, SQ_BLK], F32, tag="tr",
                              name=f"op3_{mm}")[:, 0:SQ_BLK]
            for kb in range(NPAIR):
                nc.tensor.matmul(pt, wo_sb[:, kb, mm * P:(mm + 1) * P],
                                 outT[kb][:, sqs3],
                                 start=(kb == 0), stop=(kb == NPAIR - 1))
            dst = fd3[1][:, mm - 4, :]
            if mm % 2 == 1:
                nc.scalar.copy(dst, pt)
            else:
                nc.vector.tensor_copy(dst, pt)
        nc.scalar.dma_start(ov3[:, 4:8, sqs3], fd3[1][:])

    nc.compile()
    return nc


_NC = None


def _get_nc():
    global _NC
    if _NC is None:
        _NC = _build_program()
    return _NC


def _bf(a):
    return np.ascontiguousarray(a.astype(NPBF16))


def _wre(w):
    """[K*P, N] weight -> [P, K*N] partition-major for contiguous DMA."""
    kp, n = w.shape
    k = kp // P
    return np.ascontiguousarray(
        w.reshape(k, P, n).transpose(1, 0, 2).reshape(P, k * n)
        .astype(NPBF16))


def _make_in_maps(inputs):
    q = np.asarray(inputs["query"], np.float32)
    k = np.asarray(inputs["key"], np.float32)
    v = np.asarray(inputs["value"], np.float32)
    Wq = np.asarray(inputs["Wq"], np.float32)
    Wk = np.asarray(inputs["Wk"], np.float32)
    Wv = np.asarray(inputs["Wv"], np.float32)
    Wo = np.asarray(inputs["Wo"], np.float32)
    bq = np.asarray(inputs["bq"], np.float32)
    bk = np.asarray(inputs["bk"], np.float32)
    bv = np.asarray(inputs["bv"], np.float32)
    scale = np.float32(HD ** -0.5)

    in_maps = []
    for c in range(NCORES):
        b = c // CORES_PER_BATCH
        h0 = (c % CORES_PER_BATCH) * HEADS_PER_CORE
        hsl = slice(h0 * HD, (h0 + HEADS_PER_CORE) * HD)
        bvh = bv[hsl].reshape(HEADS_PER_CORE, HD)
        bvb = np.concatenate(
            [bvh, np.ones((HEADS_PER_CORE, 1), np.float32)], axis=1).ravel()
        in_maps.append({
            "xq": _bf(q[:, b, :].T),
            "xk": _bf(k[:, b, :].T),
            "xv": _bf(v[:, b, :].T),
            "wqt": _wre((Wq[hsl, :] * scale).T),
            "wkt": _wre(Wk[hsl, :].T),
            "wvt": _wre(Wv[hsl, :].T),
            "wot": _wre(Wo[:, hsl].T),
            "bqh": np.ascontiguousarray(bq[hsl] * scale),
            "bkh": np.ascontiguousarray(bk[hsl]),
            "bvb": np.ascontiguousarray(bvb.astype(np.float32)),
            "onescr": np.ones(P, np.float32),
        })
    return in_maps


def run_sharded(inputs, trace=False):
    """Returns (full_output [S,B,E] f32, BassKernelResults)."""
    nc = _get_nc()
    in_maps = _make_in_maps(inputs)
    res = run_bass_kernel_spmd(nc, in_maps, core_ids=list(range(NCORES)),
                               trace=trace)
    bo = np.asarray(inputs["bo"], np.float32)
    final = np.zeros((S, B, E), np.float32)
    for c in range(NCORES):
        b = c // CORES_PER_BATCH
        final[:, b, :] += res.results[c]["outp"].T.astype(np.float32)
    final += bo
    return final, res


def kernel(**inputs):
    out, _ = run_sharded(inputs, trace=False)
    return out


# revision 45
# speedup vs baseline: 1.0012x; 1.0012x over previous
"""Multi-head attention (S=2048, B=2, E=1024, H=16) on 8 Trainium2 cores.

Sharding: data-parallel over batch (4 cores per batch element) x tensor-parallel
over heads (4 heads per core), Megatron-style: Wq/Wk/Wv column-sharded,
Wo row-sharded, per-core partial outputs summed (+bo) on host.

Design (v1 trace analysis showed ScalarE exp saturated at ~1.05us per
[128,1024] ACTIVATE = ~135us total; everything else is scheduled around
keeping that stream gap-free):

- qT/kT pair-packed [128, T]: partitions 0:64 = head 2p, 64:128 = head
  2p+1 (the projection PSUM layout -> single eviction per pair).
- Score matmuls row-packed via tile_position: the two K=64 head matmuls
  of a pair run concurrently in the PE array (row strips 0:64 / 64:128),
  writing the two 512-col halves (= 2 PSUM banks) of one score tile.
- Attention software-pipelined over blocks B[b] = (pair, sq) in sq-major
  order: scores+exp of block b interleave per-sk with PV of block b-1;
  normalize (sk4), o-proj (sk 7/9/11/13) of block b-2 fill the slack, so
  ScalarE runs exp back-to-back.
- Startup is DMA-critical-path ordered: wk, xk (k-proj runs k-outer
  across all 8 PSUM banks as tiles land), wq, xq[sq0] (first q-proj),
  wv, xv halves, deferred xq; v-proj and the other 6 q-proj blocks are
  interleaved into attention blocks 0-3 (ptr/ppo transient banks).
- PSUM: psc 2x[128,1024] scores + ppo 2x[65,512] PV accum (64 out rows
  + softmax-denominator row via the [v|1] ones-column trick) + ptr
  2x[128,512] transients = 8 banks.
- Normalization: denominator rows -> reciprocal -> K=1 broadcast matmul
  -> outT = outU * bcast(1/den), merged per block.
- Output written bf16 (f32 partials summed on host); drain o-proj spreads
  over 4 PSUM slots, DVE + ScalarE fin casts, and both HWDGE DMA queues.

Numerics: softmax without max-subtraction (scores ~ N(0,1), exp safe in
bf16 range); exp@[v|1] then divide == softmax@v; bq,bk folded into
projection evictions; bv added to v; bo added on host; matmuls bf16,
PSUM accumulation fp32. Measured vs fp32 reference: rel err ~7.1e-3.
"""

import numpy as np
import ml_dtypes
from contextlib import ExitStack

import concourse.bass as bass
import concourse.mybir as mybir
from concourse import bacc
import concourse.tile as tile
from concourse.bass_utils import run_bass_kernel_spmd

S, B, E, H, HD = 2048, 2, 1024, 16, 64
P = 128
NCORES = 8
CORES_PER_BATCH = 4
HEADS_PER_CORE = H // CORES_PER_BATCH      # 4
LOCAL_E = HEADS_PER_CORE * HD              # 256
VW = HD + 1                                # 65: [v_h | ones]
T = S                                      # tokens per core (one batch elem)
KT = E // P                                # 8 contraction tiles for projections
NPAIR = HEADS_PER_CORE // 2                # 2 head pairs
SQ_BLK = 512
NSQ = T // SQ_BLK                          # 4
NSK = T // P                               # 16
NBLK = NPAIR * NSQ                         # 8 attention blocks
F32 = mybir.dt.float32
F32R = mybir.dt.float32r
BF16 = mybir.dt.bfloat16
NPBF16 = ml_dtypes.bfloat16
EXPF = mybir.ActivationFunctionType.Exp


def _build_program():
    nc = bacc.Bacc("TRN2")

    xq = nc.dram_tensor("xq", [E, T], BF16, kind="ExternalInput")
    xk = nc.dram_tensor("xk", [E, T], BF16, kind="ExternalInput")
    xv = nc.dram_tensor("xv", [E, T], BF16, kind="ExternalInput")
    # weights pre-rearranged on host to [P, k, n] partition-major layouts
    wqt = nc.dram_tensor("wqt", [P, KT * LOCAL_E], BF16,
                         kind="ExternalInput")
    wkt = nc.dram_tensor("wkt", [P, KT * LOCAL_E], BF16,
                         kind="ExternalInput")
    wvt = nc.dram_tensor("wvt", [P, KT * LOCAL_E], BF16,
                         kind="ExternalInput")
    wot = nc.dram_tensor("wot", [P, (LOCAL_E // P) * E], BF16,
                         kind="ExternalInput")
    bqh = nc.dram_tensor("bqh", [LOCAL_E], F32, kind="ExternalInput")
    bkh = nc.dram_tensor("bkh", [LOCAL_E], F32, kind="ExternalInput")
    # per head: [bv_head (64), 1.0] -> 65 columns
    bvb = nc.dram_tensor("bvb", [HEADS_PER_CORE * VW], F32R,
                         kind="ExternalInput")
    onescr = nc.dram_tensor("onescr", [P], F32R, kind="ExternalInput")
    outp = nc.dram_tensor("outp", [E, T], BF16, kind="ExternalOutput")

    with ExitStack() as ctx:
        ctx.enter_context(nc.allow_low_precision(reason="bf16 matmul pipeline"))
        tc = ctx.enter_context(tile.TileContext(nc))
        xpool = ctx.enter_context(tc.tile_pool(name="xpool", bufs=8))
        wpool = ctx.enter_context(tc.tile_pool(name="wpool", bufs=1))
        qkpool = ctx.enter_context(tc.tile_pool(name="qkpool", bufs=4))
        vpool = ctx.enter_context(tc.tile_pool(name="vpool", bufs=1))
        opool = ctx.enter_context(tc.tile_pool(name="opool", bufs=2))
        epool = ctx.enter_context(tc.tile_pool(name="epool", bufs=18))
        fpool = ctx.enter_context(tc.tile_pool(name="fpool", bufs=4))
        spool = ctx.enter_context(tc.tile_pool(name="spool", bufs=4))
        cpool = ctx.enter_context(tc.tile_pool(name="cpool", bufs=1))
        # PSUM: psc 2x[128,1024] = 4 banks; po 2x[65,512] = 2 banks;
        #       ptr 2x[128,512] transient (bcast/o-proj/v-proj) = 2 banks.
        psc = ctx.enter_context(tc.tile_pool(name="psc", bufs=2, space="PSUM"))
        ppo = ctx.enter_context(tc.tile_pool(name="ppo", bufs=2, space="PSUM"))
        ptr = ctx.enter_context(tc.tile_pool(name="ptr", bufs=2, space="PSUM"))

        # ---- constants + weights (DMA order matters: wk then xk first) --
        onesr_sb = cpool.tile([1, P], F32R, tag="onesr")
        nc.sync.dma_start(onesr_sb[:], onescr[None, :])
        bvb_sb = cpool.tile([1, HEADS_PER_CORE * VW], F32R, tag="bvbrow")
        nc.sync.dma_start(bvb_sb[:], bvb[None, :])
        # per-pair bias columns [128, NPAIR]
        bq_sb = cpool.tile([P, NPAIR], F32, tag="bq")
        nc.sync.dma_start(bq_sb[:], bqh.rearrange("(m p) -> p m", p=P))
        bk_sb = cpool.tile([P, NPAIR], F32, tag="bk")
        nc.sync.dma_start(bk_sb[:], bkh.rearrange("(m p) -> p m", p=P))

        # DMA priority order: wk+xk (k-proj), wq+xq_n0 (first q-proj),
        # wv+xv (v-proj in blocks 0-1), xq_n1-3 (deferred q-proj), wo
        HT = T // 2
        wk_sb = wpool.tile([P, KT, LOCAL_E], BF16, tag="wk")
        nc.sync.dma_start(wk_sb.rearrange("p k n -> p (k n)"), wkt[:, :])
        xk_t = [xpool.tile([P, T], BF16, tag="xk", name=f"xk{k}")
                for k in range(KT)]
        for k in range(KT):
            nc.sync.dma_start(xk_t[k][:, 0:HT], xk[k * P:(k + 1) * P, 0:HT])
        wq_sb = wpool.tile([P, KT, LOCAL_E], BF16, tag="wq")
        nc.sync.dma_start(wq_sb.rearrange("p k n -> p (k n)"), wqt[:, :])
        xq_t = [xpool.tile([P, T], BF16, tag="xq", name=f"xq{k}")
                for k in range(KT)]
        for k in range(KT):
            nc.sync.dma_start(xq_t[k][:, 0:SQ_BLK],
                              xq[k * P:(k + 1) * P, 0:SQ_BLK])
        for k in range(KT):
            nc.sync.dma_start(xk_t[k][:, HT:T], xk[k * P:(k + 1) * P, HT:T])
        wv_sb = wpool.tile([P, KT, LOCAL_E], BF16, tag="wv")
        nc.sync.dma_start(wv_sb.rearrange("p k n -> p (k n)"), wvt[:, :])
        xv_t = [xpool.tile([P, T], BF16, tag="xv", name=f"xv{k}")
                for k in range(KT)]
        for half in range(2):
            for k in range(KT):
                nc.sync.dma_start(xv_t[k][:, half * HT:(half + 1) * HT],
                                  xv[k * P:(k + 1) * P,
                                     half * HT:(half + 1) * HT])
        for n in range(1, NSQ):
            for k in range(KT):
                nc.sync.dma_start(
                    xq_t[k][:, n * SQ_BLK:(n + 1) * SQ_BLK],
                    xq[k * P:(k + 1) * P, n * SQ_BLK:(n + 1) * SQ_BLK])
        wo_sb = wpool.tile([P, LOCAL_E // P, E], BF16, tag="wo")
        nc.sync.dma_start(wo_sb.rearrange("p k n -> p (k n)"), wot[:, :])

        # preload the exp table-set during the projection phase
        warm = spool.tile([1, 2], F32, tag="warm")
        nc.scalar.activation(warm[0:1, 0:2], bq_sb[0:1, 0:2], EXPF)

        # broadcast [bv_h | 1] over all 128 partitions via a K=1 outer product
        bvb_ps = ptr.tile([P, SQ_BLK], F32, tag="tr")
        nc.tensor.matmul(bvb_ps[:, 0:HEADS_PER_CORE * VW],
                         onesr_sb[0:1, :], bvb_sb[0:1, :],
                         start=True, stop=True)
        bvb_bc = cpool.tile([P, HEADS_PER_CORE * VW], F32, tag="bvbbc")
        nc.vector.tensor_copy(bvb_bc[:], bvb_ps[:, 0:HEADS_PER_CORE * VW])

        # ---- persistent activations -----------------------------------
        # pair-packed: partitions 0:64 = head 2m, 64:128 = head 2m+1
        qT = [qkpool.tile([P, T], BF16, tag="qk", name=f"qT{m}")
              for m in range(NPAIR)]
        kT = [qkpool.tile([P, T], BF16, tag="qk", name=f"kT{m}")
              for m in range(NPAIR)]
        # v buffer: per sk-tile, per head: [v_h (64 cols) | ones (1 col)]
        vbuf = vpool.tile([P, NSK, HEADS_PER_CORE * VW], BF16, tag="v")
        for tt in range(NSK):
            nc.vector.tensor_copy(
                vbuf.rearrange("p s (h c) -> p s h c", c=VW)
                [:, tt, :, HD:HD + 1],
                bvb_bc.rearrange("p (h c) -> p h c", c=VW)[:, :, HD:HD + 1])
        # normalized attention outputs per pair [128, T] (pair-packed rows)
        outT = [opool.tile([P, T], BF16, tag="oT", name=f"outT{m}")
                for m in range(NPAIR)]

        # ---- Q/K projections: pair-packed eviction --------------------
        def qk_proj_blk(xt, w_sb, bias_sb, dsts, n, m, pool):
            if pool is psc:
                ps = psc.tile([P, 2 * SQ_BLK], F32, tag="sc",
                              name=f"qkp{n}_{m}")
            else:
                ps = ptr.tile([P, SQ_BLK], F32, tag="tr",
                              name=f"qkp{n}_{m}")
            use_scalar = pool is psc
            for k in range(KT):
                nc.tensor.matmul(
                    ps[:, 0:SQ_BLK],
                    w_sb[:, k, m * P:(m + 1) * P],
                    xt[k][:, n * SQ_BLK:(n + 1) * SQ_BLK],
                    start=(k == 0), stop=(k == KT - 1))
            if use_scalar:
                nc.scalar.add(
                    dsts[m][:, n * SQ_BLK:(n + 1) * SQ_BLK],
                    ps[:, 0:SQ_BLK], bias_sb[:, m:m + 1])
            else:
                nc.vector.tensor_scalar_add(
                    dsts[m][:, n * SQ_BLK:(n + 1) * SQ_BLK],
                    ps[:, 0:SQ_BLK], bias_sb[:, m:m + 1])

        # upfront K-proj covers only token-half 0 (sq blocks 0,1) so the
        # first scores wait on 2MB of xk + 1MB of xq instead of all 4MB;
        # token-half 1 (pass B) and q-proj(m1) are deferred into block 0
        kpsA = psc.tile([P, 2 * SQ_BLK], F32, tag="sc", name="kpsA")
        kps1t = [ptr.tile([P, SQ_BLK], F32, tag="tr", name=f"kpA1_{n}")
                 for n in range(2)]
        for k in range(KT):
            for n in range(2):
                nc.tensor.matmul(
                    kpsA[:, n * SQ_BLK:(n + 1) * SQ_BLK],
                    wk_sb[:, k, 0:P],
                    xk_t[k][:, n * SQ_BLK:(n + 1) * SQ_BLK],
                    start=(k == 0), stop=(k == KT - 1))
            for n in range(2):
                nc.tensor.matmul(
                    kps1t[n][:, 0:SQ_BLK],
                    wk_sb[:, k, P:2 * P],
                    xk_t[k][:, n * SQ_BLK:(n + 1) * SQ_BLK],
                    start=(k == 0), stop=(k == KT - 1))
        for n in range(2):
            nc.scalar.add(kT[0][:, n * SQ_BLK:(n + 1) * SQ_BLK],
                          kpsA[:, n * SQ_BLK:(n + 1) * SQ_BLK],
                          bk_sb[:, 0:1])
        for n in range(2):
            nc.vector.tensor_scalar_add(
                kT[1][:, n * SQ_BLK:(n + 1) * SQ_BLK],
                kps1t[n][:, 0:SQ_BLK], bk_sb[:, 1:2])
        qk_proj_blk(xq_t, wq_sb, bq_sb, qT, 0, 0, psc)

        def kproj_passB(m):
            ts = [ptr.tile([P, SQ_BLK], F32, tag="tr", name=f"kpB{m}_{n}")
                  for n in range(2)]
            for k in range(KT):
                for n in range(2):
                    nc.tensor.matmul(
                        ts[n][:, 0:SQ_BLK],
                        wk_sb[:, k, m * P:(m + 1) * P],
                        xk_t[k][:, (2 + n) * SQ_BLK:(3 + n) * SQ_BLK],
                        start=(k == 0), stop=(k == KT - 1))
            for n in range(2):
                nc.vector.tensor_scalar_add(
                    kT[m][:, (2 + n) * SQ_BLK:(3 + n) * SQ_BLK],
                    ts[n][:, 0:SQ_BLK], bk_sb[:, m:m + 1])

        # ---- V projection: v[t, o] = (xT[k,t]).T @ WvT[k, o] (+ bv) ----
        # deferred into attention blocks 0-1 (xv DMA lands mid-block 0)
        def v_proj(tt):
            ps = ptr.tile([P, SQ_BLK], F32, tag="tr", name=f"vps{tt}")
            for k in range(KT):
                nc.tensor.matmul(
                    ps[:, 0:LOCAL_E],
                    xv_t[k][:, tt * P:(tt + 1) * P],
                    wv_sb[:, k, :],
                    start=(k == 0), stop=(k == KT - 1))
            nc.vector.tensor_tensor(
                vbuf.rearrange("p s (h c) -> p s h c", c=VW)[:, tt, :, 0:HD],
                ps[:, 0:LOCAL_E].rearrange("p (h c) -> p h c", c=HD),
                bvb_bc.rearrange("p (h c) -> p h c", c=VW)[:, :, 0:HD],
                mybir.AluOpType.add)

        # ---- software-pipelined attention -----------------------------
        # sq-major block order: output projection of sq can run (and its
        # DMA drain) two blocks after (pr1, sq) completes
        BLKS = [(m, sq) for sq in range(NSQ) for m in range(NPAIR)]
        # deferred q-projection blocks: (bi, sk) -> (n, m); deadline for
        # qT[m][:, n] is block 2n+m (sq-major)
        QPROJ_SCHED = {(1, 10): (1, 0), (1, 15): (1, 1),
                       (2, 2): (2, 0), (2, 15): (2, 1),
                       (4, 2): (3, 0), (4, 10): (3, 1)}

        exs = {}       # b -> list of 16 ex tiles
        po_t = {}      # b -> [po0, po1]
        outU_t = {}    # b -> [128, SQ_BLK] unnormalized (pair-packed rows)
        dn_t = {}      # b -> [2, SQ_BLK] denominators

        FE_A = 128.0 * 1.4426950408889634          # 2^7 * log2(e)
        FE_B = 127.0 * 128.0 - 128.0 * 0.0573 + 0.5  # bias + err-center + rnd

        def em_scores_exp(b, sk, dve=False):
            m, sq = BLKS[b]
            sqs = slice(sq * SQ_BLK, (sq + 1) * SQ_BLK)
            sks = slice(sk * P, (sk + 1) * P)
            ps = psc.tile([P, 2 * SQ_BLK], F32, tag="sc",
                          name=f"sc{b}_{sk}")
            # two K=64 head matmuls packed as PE row-tiles (0,0) and (64,0)
            nc.tensor.matmul(ps[:, 0:SQ_BLK],
                             kT[m][0:HD, sks], qT[m][0:HD, sqs],
                             start=True, stop=True)
            nc.tensor.matmul(ps[:, SQ_BLK:2 * SQ_BLK],
                             kT[m][HD:P, sks], qT[m][HD:P, sqs],
                             start=True, stop=True)
            ex = epool.tile([P, 2 * SQ_BLK], BF16, tag="e",
                            name=f"ex{b}_{sk}")
            if dve:
                # fast-exp bit trick on the (otherwise idle) vector engine:
                # int16(s*FE_A + FE_B) bitcast as bf16 ~ exp(s), +-4.3%
                nc.vector.tensor_scalar(
                    ex[:].bitcast(mybir.dt.int16), ps[:], FE_A, FE_B,
                    mybir.AluOpType.mult, mybir.AluOpType.add)
            else:
                nc.scalar.activation(ex[:], ps[:], EXPF)
            exs[b].append(ex)

        def em_pv(b, sk):
            m, sq = BLKS[b]
            ex = exs[b][sk]
            for X in range(2):
                h = 2 * m + X
                nc.tensor.matmul(
                    po_t[b][X][0:VW, :],
                    vbuf[:, sk, h * VW:(h + 1) * VW],
                    ex[:, X * SQ_BLK:(X + 1) * SQ_BLK],
                    start=(sk == 0), stop=(sk == NSK - 1))

        def em_evict_po(b, drain=False):
            # right after PV(b, 15): free the two po banks quickly; in the
            # drain the scalar engine is idle and takes the outU copies
            oU = spool.tile([P, SQ_BLK], BF16, tag="oU", bufs=2,
                            name=f"oU{b}")
            dn = [spool.tile([1, SQ_BLK], F32, tag="dn", bufs=2,
                             name=f"dn{b}_{X}") for X in range(2)]
            for X in range(2):
                if drain:
                    nc.scalar.copy(oU[X * HD:(X + 1) * HD, :],
                                   po_t[b][X][0:HD, :])
                else:
                    nc.vector.tensor_copy(oU[X * HD:(X + 1) * HD, :],
                                          po_t[b][X][0:HD, :])
                nc.vector.tensor_copy(dn[X][:], po_t[b][X][HD:VW, :])
            outU_t[b] = oU
            dn_t[b] = dn

        bc_t = {}

        def em_normalize_x(b, X, drain=False):
            if X == 0:
                bc_t[b] = spool.tile([P, SQ_BLK], BF16, tag="bc", bufs=2,
                                     name=f"bc{b}")
            rc = spool.tile([1, SQ_BLK], F32, tag="rc", bufs=2,
                            name=f"rc{b}_{X}")
            nc.vector.reciprocal_approx_fast(rc[:], dn_t[b][X][:])
            rcr = spool.tile([1, SQ_BLK], F32R, tag="rcr", bufs=2,
                             name=f"rcr{b}_{X}")
            nc.vector.tensor_copy(rcr[:], rc[:])
            pb = ptr.tile([P, SQ_BLK], F32, tag="tr",
                          name=f"pb{b}_{X}")
            nc.tensor.matmul(pb[0:HD, :], onesr_sb[0:1, 0:HD], rcr[:],
                             start=True, stop=True)
            if drain:
                nc.scalar.copy(bc_t[b][X * HD:(X + 1) * HD, :], pb[0:HD, :])
            else:
                nc.vector.tensor_copy(bc_t[b][X * HD:(X + 1) * HD, :],
                                      pb[0:HD, :])

        def em_normalize_fin(b):
            m, sq = BLKS[b]
            sqs = slice(sq * SQ_BLK, (sq + 1) * SQ_BLK)
            nc.vector.tensor_tensor(outT[m][:, sqs], outU_t[b][:],
                                    bc_t[b][:], mybir.AluOpType.mult)

        fd_t = {}

        def em_oproj(sq, m_range, drain=False):
            sqs = slice(sq * SQ_BLK, (sq + 1) * SQ_BLK)
            if m_range[0] == 0:
                # batch output into 2 wide DMAs per sq (8 chunk-descriptors
                # pay ~1.2us completion each on the queue)
                fd_t[sq] = [fpool.tile([P, 4, SQ_BLK], BF16, tag="fd",
                                       bufs=2, name=f"fd{sq}_{g}")
                            for g in range(2)]
            fd = fd_t[sq]
            ov = outp.rearrange("(g p) t -> p g t", p=P)
            for mm in m_range:
                # in the drain, spread across 4 PSUM slots and both the DVE
                # and the (now idle) scalar engine / scalar DMA queue
                if drain and mm % 2 == 1:
                    pt = ppo.tile([P, SQ_BLK], F32, tag="po",
                                  name=f"op{sq}_{mm}")
                else:
                    pt = ptr.tile([P, SQ_BLK], F32, tag="tr",
                                  name=f"op{sq}_{mm}")
                for kb in range(NPAIR):
                    nc.tensor.matmul(
                        pt[:],
                        wo_sb[:, kb, mm * P:(mm + 1) * P],
                        outT[kb][:, sqs],
                        start=(kb == 0), stop=(kb == NPAIR - 1))
                dst = fd[mm // 4][:, mm % 4, :]
                if drain and mm % 2 == 1:
                    nc.scalar.copy(dst, pt[:])
                else:
                    nc.vector.tensor_copy(dst, pt[:])
                if mm == 3:
                    nc.sync.dma_start(ov[:, 0:4, sqs], fd[0][:])
                if mm == 7:
                    if drain:
                        nc.scalar.dma_start(ov[:, 4:8, sqs], fd[1][:])
                    else:
                        nc.sync.dma_start(ov[:, 4:8, sqs], fd[1][:])

        for bi in range(NBLK + 1):
            drain = bi == NBLK
            if not drain:
                exs[bi] = []
                po_t[bi] = [ppo.tile([VW, SQ_BLK], F32, tag="po",
                                     name=f"po{bi}_{X}") for X in range(2)]
            for sk in range(NSK):
                if bi >= 1:
                    em_pv(bi - 1, sk)
                if not drain:
                    # note: offloading some tiles' exp to the DVE via the
                    # fast-exp bit trick (dve=True) is numerically fine
                    # (rel err ~9e-3) but measured net-neutral-to-worse:
                    # the DVE's bursty FIFO delivers the tiles late and the
                    # psc WAR then stalls the PE score stream.
                    em_scores_exp(bi, sk)
                # deferred startup work in block 0's slack: k-proj token
                # half 1, q-proj (sq0, m1), then the v-projection
                if bi == 0 and sk == 0:
                    kproj_passB(0)
                if bi == 0 and sk == 2:
                    qk_proj_blk(xq_t, wq_sb, bq_sb, qT, 0, 1, psc)
                if bi == 0 and sk == 4:
                    kproj_passB(1)
                if bi == 0 and sk >= 9:
                    v_proj(sk - 9)
                if bi == 1 and sk < 9:
                    v_proj(sk + 7)
                if (bi, sk) in QPROJ_SCHED:
                    n, m = QPROJ_SCHED[(bi, sk)]
                    qk_proj_blk(xq_t, wq_sb, bq_sb, qT, n, m, ptr)
                if bi >= 1 and sk == NSK - 1:
                    em_evict_po(bi - 1, drain=drain)
                if bi >= 2:
                    # stagger block bi-2's post-processing into this block
                    nb = bi - 2
                    if sk == 4:
                        em_normalize_x(nb, 0)
                        em_normalize_x(nb, 1)
                        em_normalize_fin(nb)
                    # o-proj of sq once both its pair-blocks are normalized
                    if bi >= 3 and bi % 2 == 1:
                        osq = (bi - 3) // 2
                        if sk in (7, 9, 11, 13):
                            i0 = (sk - 7) // 2 * 2
                            em_oproj(osq, range(i0, i0 + 2))
        # tail: the last block's normalize + o-proj for the last sq
        em_normalize_x(NBLK - 1, 0)
        em_normalize_x(NBLK - 1, 1)
        em_normalize_fin(NBLK - 1)
        em_oproj(NSQ - 1, range(0, 8), drain=True)

    nc.compile()
    return nc


_NC = None


def _get_nc():
    global _NC
    if _NC is None:
        _NC = _build_program()
    return _NC


def _bf(a):
    return np.ascontiguousarray(a.astype(NPBF16))


def _wre(w):
    """[K*P, N] weight -> [P, K*N] partition-major for contiguous DMA."""
    kp, n = w.shape
    k = kp // P
    return np.ascontiguousarray(
        w.reshape(k, P, n).transpose(1, 0, 2).reshape(P, k * n)
        .astype(NPBF16))


def _make_in_maps(inputs):
    q = np.asarray(inputs["query"], np.float32)
    k = np.asarray(inputs["key"], np.float32)
    v = np.asarray(inputs["value"], np.float32)
    Wq = np.asarray(inputs["Wq"], np.float32)
    Wk = np.asarray(inputs["Wk"], np.float32)
    Wv = np.asarray(inputs["Wv"], np.float32)
    Wo = np.asarray(inputs["Wo"], np.float32)
    bq = np.asarray(inputs["bq"], np.float32)
    bk = np.asarray(inputs["bk"], np.float32)
    bv = np.asarray(inputs["bv"], np.float32)
    scale = np.float32(HD ** -0.5)

    in_maps = []
    for c in range(NCORES):
        b = c // CORES_PER_BATCH
        h0 = (c % CORES_PER_BATCH) * HEADS_PER_CORE
        hsl = slice(h0 * HD, (h0 + HEADS_PER_CORE) * HD)
        bvh = bv[hsl].reshape(HEADS_PER_CORE, HD)
        bvb = np.concatenate(
            [bvh, np.ones((HEADS_PER_CORE, 1), np.float32)], axis=1).ravel()
        in_maps.append({
            "xq": _bf(q[:, b, :].T),
            "xk": _bf(k[:, b, :].T),
            "xv": _bf(v[:, b, :].T),
            "wqt": _wre((Wq[hsl, :] * scale).T),
            "wkt": _wre(Wk[hsl, :].T),
            "wvt": _wre(Wv[hsl, :].T),
            "wot": _wre(Wo[:, hsl].T),
            "bqh": np.ascontiguousarray(bq[hsl] * scale),
            "bkh": np.ascontiguousarray(bk[hsl]),
            "bvb": np.ascontiguousarray(bvb.astype(np.float32)),
            "onescr": np.ones(P, np.float32),
        })
    return in_maps


def run_sharded(inputs, trace=False):
    """Returns (full_output [S,B,E] f32, BassKernelResults)."""
    nc = _get_nc()
    in_maps = _make_in_maps(inputs)
    res = run_bass_kernel_spmd(nc, in_maps, core_ids=list(range(NCORES)),
                               trace=trace)
    bo = np.asarray(inputs["bo"], np.float32)
    final = np.zeros((S, B, E), np.float32)
    for c in range(NCORES):
        b = c // CORES_PER_BATCH
        final[:, b, :] += res.results[c]["outp"].T.astype(np.float32)
    final += bo
    return final, res


def kernel(**inputs):
    out, _ = run_sharded(inputs, trace=False)
    return out


# revision 46
# speedup vs baseline: 1.0037x; 1.0026x over previous
"""Multi-head attention (S=2048, B=2, E=1024, H=16) on 8 Trainium2 cores.

Sharding: data-parallel over batch (4 cores per batch element) x tensor-parallel
over heads (4 heads per core), Megatron-style: Wq/Wk/Wv column-sharded,
Wo row-sharded, per-core partial outputs summed (+bo) on host.

Design (v1 trace analysis showed ScalarE exp saturated at ~1.05us per
[128,1024] ACTIVATE = ~135us total; everything else is scheduled around
keeping that stream gap-free):

- qT/kT pair-packed [128, T]: partitions 0:64 = head 2p, 64:128 = head
  2p+1 (the projection PSUM layout -> single eviction per pair).
- Score matmuls row-packed via tile_position: the two K=64 head matmuls
  of a pair run concurrently in the PE array (row strips 0:64 / 64:128),
  writing the two 512-col halves (= 2 PSUM banks) of one score tile.
- Attention software-pipelined over blocks B[b] = (pair, sq) in sq-major
  order: scores+exp of block b interleave per-sk with PV of block b-1;
  normalize (sk4), o-proj (sk 7/9/11/13) of block b-2 fill the slack, so
  ScalarE runs exp back-to-back.
- Startup is DMA-critical-path ordered: wk, xk (k-proj runs k-outer
  across all 8 PSUM banks as tiles land), wq, xq[sq0] (first q-proj),
  wv, xv halves, deferred xq; v-proj and the other 6 q-proj blocks are
  interleaved into attention blocks 0-3 (ptr/ppo transient banks).
- PSUM: psc 2x[128,1024] scores + ppo 2x[65,512] PV accum (64 out rows
  + softmax-denominator row via the [v|1] ones-column trick) + ptr
  2x[128,512] transients = 8 banks.
- Normalization: denominator rows -> reciprocal -> K=1 broadcast matmul
  -> outT = outU * bcast(1/den), merged per block.
- Output written bf16 (f32 partials summed on host); drain o-proj spreads
  over 4 PSUM slots, DVE + ScalarE fin casts, and both HWDGE DMA queues.

Numerics: softmax without max-subtraction (scores ~ N(0,1), exp safe in
bf16 range); exp@[v|1] then divide == softmax@v; bq,bk folded into
projection evictions; bv added to v; bo added on host; matmuls bf16,
PSUM accumulation fp32. Measured vs fp32 reference: rel err ~7.1e-3.
"""

import numpy as np
import ml_dtypes
from contextlib import ExitStack

import concourse.bass as bass
import concourse.mybir as mybir
from concourse import bacc
import concourse.tile as tile
from concourse.bass_utils import run_bass_kernel_spmd

S, B, E, H, HD = 2048, 2, 1024, 16, 64
P = 128
NCORES = 8
CORES_PER_BATCH = 4
HEADS_PER_CORE = H // CORES_PER_BATCH      # 4
LOCAL_E = HEADS_PER_CORE * HD              # 256
VW = HD + 1                                # 65: [v_h | ones]
T = S                                      # tokens per core (one batch elem)
KT = E // P                                # 8 contraction tiles for projections
NPAIR = HEADS_PER_CORE // 2                # 2 head pairs
SQ_BLK = 512
NSQ = T // SQ_BLK                          # 4
NSK = T // P                               # 16
NBLK = NPAIR * NSQ                         # 8 attention blocks
F32 = mybir.dt.float32
F32R = mybir.dt.float32r
BF16 = mybir.dt.bfloat16
NPBF16 = ml_dtypes.bfloat16
EXPF = mybir.ActivationFunctionType.Exp


def _build_program():
    nc = bacc.Bacc("TRN2")

    xq = nc.dram_tensor("xq", [E, T], BF16, kind="ExternalInput")
    xk = nc.dram_tensor("xk", [E, T], BF16, kind="ExternalInput")
    xv = nc.dram_tensor("xv", [E, T], BF16, kind="ExternalInput")
    # weights pre-rearranged on host to [P, k, n] partition-major layouts
    wqt = nc.dram_tensor("wqt", [P, KT * LOCAL_E], BF16,
                         kind="ExternalInput")
    wkt = nc.dram_tensor("wkt", [P, KT * LOCAL_E], BF16,
                         kind="ExternalInput")
    wvt = nc.dram_tensor("wvt", [P, KT * LOCAL_E], BF16,
                         kind="ExternalInput")
    wot = nc.dram_tensor("wot", [P, (LOCAL_E // P) * E], BF16,
                         kind="ExternalInput")
    bqh = nc.dram_tensor("bqh", [LOCAL_E], F32, kind="ExternalInput")
    bkh = nc.dram_tensor("bkh", [LOCAL_E], F32, kind="ExternalInput")
    # per head: [bv_head (64), 1.0] -> 65 columns
    bvb = nc.dram_tensor("bvb", [HEADS_PER_CORE * VW], F32R,
                         kind="ExternalInput")
    onescr = nc.dram_tensor("onescr", [P], F32R, kind="ExternalInput")
    outp = nc.dram_tensor("outp", [E, T], BF16, kind="ExternalOutput")

    with ExitStack() as ctx:
        ctx.enter_context(nc.allow_low_precision(reason="bf16 matmul pipeline"))
        tc = ctx.enter_context(tile.TileContext(nc))
        xpool = ctx.enter_context(tc.tile_pool(name="xpool", bufs=8))
        wpool = ctx.enter_context(tc.tile_pool(name="wpool", bufs=1))
        qkpool = ctx.enter_context(tc.tile_pool(name="qkpool", bufs=4))
        vpool = ctx.enter_context(tc.tile_pool(name="vpool", bufs=1))
        opool = ctx.enter_context(tc.tile_pool(name="opool", bufs=2))
        epool = ctx.enter_context(tc.tile_pool(name="epool", bufs=18))
        fpool = ctx.enter_context(tc.tile_pool(name="fpool", bufs=4))
        spool = ctx.enter_context(tc.tile_pool(name="spool", bufs=4))
        cpool = ctx.enter_context(tc.tile_pool(name="cpool", bufs=1))
        # PSUM: psc 2x[128,1024] = 4 banks; po 2x[65,512] = 2 banks;
        #       ptr 2x[128,512] transient (bcast/o-proj/v-proj) = 2 banks.
        psc = ctx.enter_context(tc.tile_pool(name="psc", bufs=2, space="PSUM"))
        ppo = ctx.enter_context(tc.tile_pool(name="ppo", bufs=2, space="PSUM"))
        ptr = ctx.enter_context(tc.tile_pool(name="ptr", bufs=2, space="PSUM"))

        # ---- constants + weights (DMA order matters: wk then xk first) --
        onesr_sb = cpool.tile([1, P], F32R, tag="onesr")
        nc.sync.dma_start(onesr_sb[:], onescr[None, :])
        bvb_sb = cpool.tile([1, HEADS_PER_CORE * VW], F32R, tag="bvbrow")
        nc.sync.dma_start(bvb_sb[:], bvb[None, :])
        # per-pair bias columns [128, NPAIR]
        bq_sb = cpool.tile([P, NPAIR], F32, tag="bq")
        nc.sync.dma_start(bq_sb[:], bqh.rearrange("(m p) -> p m", p=P))
        bk_sb = cpool.tile([P, NPAIR], F32, tag="bk")
        nc.sync.dma_start(bk_sb[:], bkh.rearrange("(m p) -> p m", p=P))

        # DMA priority order: wk+xk (k-proj), wq+xq_n0 (first q-proj),
        # wv+xv (v-proj in blocks 0-1), xq_n1-3 (deferred q-proj), wo
        HT = T // 2
        wk_sb = wpool.tile([P, KT, LOCAL_E], BF16, tag="wk")
        nc.sync.dma_start(wk_sb.rearrange("p k n -> p (k n)"), wkt[:, :])
        xk_t = [xpool.tile([P, T], BF16, tag="xk", name=f"xk{k}")
                for k in range(KT)]
        for k in range(KT):
            nc.sync.dma_start(xk_t[k][:, 0:HT], xk[k * P:(k + 1) * P, 0:HT])
        wq_sb = wpool.tile([P, KT, LOCAL_E], BF16, tag="wq")
        nc.sync.dma_start(wq_sb.rearrange("p k n -> p (k n)"), wqt[:, :])
        xq_t = [xpool.tile([P, T], BF16, tag="xq", name=f"xq{k}")
                for k in range(KT)]
        for k in range(KT):
            nc.sync.dma_start(xq_t[k][:, 0:SQ_BLK],
                              xq[k * P:(k + 1) * P, 0:SQ_BLK])
        for k in range(KT):
            nc.sync.dma_start(xk_t[k][:, HT:T], xk[k * P:(k + 1) * P, HT:T])
        wv_sb = wpool.tile([P, KT, LOCAL_E], BF16, tag="wv")
        nc.sync.dma_start(wv_sb.rearrange("p k n -> p (k n)"), wvt[:, :])
        xv_t = [xpool.tile([P, T], BF16, tag="xv", name=f"xv{k}")
                for k in range(KT)]
        for half in range(2):
            for k in range(KT):
                nc.sync.dma_start(xv_t[k][:, half * HT:(half + 1) * HT],
                                  xv[k * P:(k + 1) * P,
                                     half * HT:(half + 1) * HT])
        for n in range(1, NSQ):
            for k in range(KT):
                nc.sync.dma_start(
                    xq_t[k][:, n * SQ_BLK:(n + 1) * SQ_BLK],
                    xq[k * P:(k + 1) * P, n * SQ_BLK:(n + 1) * SQ_BLK])
        wo_sb = wpool.tile([P, LOCAL_E // P, E], BF16, tag="wo")
        nc.sync.dma_start(wo_sb.rearrange("p k n -> p (k n)"), wot[:, :])

        # preload the exp table-set during the projection phase
        warm = spool.tile([1, 2], F32, tag="warm")
        nc.scalar.activation(warm[0:1, 0:2], bq_sb[0:1, 0:2], EXPF)

        # broadcast [bv_h | 1] over all 128 partitions via a K=1 outer product
        bvb_ps = ptr.tile([P, SQ_BLK], F32, tag="tr")
        nc.tensor.matmul(bvb_ps[:, 0:HEADS_PER_CORE * VW],
                         onesr_sb[0:1, :], bvb_sb[0:1, :],
                         start=True, stop=True)
        bvb_bc = cpool.tile([P, HEADS_PER_CORE * VW], F32, tag="bvbbc")
        nc.vector.tensor_copy(bvb_bc[:], bvb_ps[:, 0:HEADS_PER_CORE * VW])

        # ---- persistent activations -----------------------------------
        # pair-packed: partitions 0:64 = head 2m, 64:128 = head 2m+1
        qT = [qkpool.tile([P, T], BF16, tag="qk", name=f"qT{m}")
              for m in range(NPAIR)]
        kT = [qkpool.tile([P, T], BF16, tag="qk", name=f"kT{m}")
              for m in range(NPAIR)]
        # v buffer: per sk-tile, per head: [v_h (64 cols) | ones (1 col)]
        vbuf = vpool.tile([P, NSK, HEADS_PER_CORE * VW], BF16, tag="v")
        for tt in range(NSK):
            nc.vector.tensor_copy(
                vbuf.rearrange("p s (h c) -> p s h c", c=VW)
                [:, tt, :, HD:HD + 1],
                bvb_bc.rearrange("p (h c) -> p h c", c=VW)[:, :, HD:HD + 1])
        # normalized attention outputs per pair [128, T] (pair-packed rows)
        outT = [opool.tile([P, T], BF16, tag="oT", name=f"outT{m}")
                for m in range(NPAIR)]

        # ---- Q/K projections: pair-packed eviction --------------------
        def qk_proj_blk(xt, w_sb, bias_sb, dsts, n, m, pool):
            if pool is psc:
                ps = psc.tile([P, 2 * SQ_BLK], F32, tag="sc",
                              name=f"qkp{n}_{m}")
            else:
                ps = ptr.tile([P, SQ_BLK], F32, tag="tr",
                              name=f"qkp{n}_{m}")
            use_scalar = pool is psc
            for k in range(KT):
                nc.tensor.matmul(
                    ps[:, 0:SQ_BLK],
                    w_sb[:, k, m * P:(m + 1) * P],
                    xt[k][:, n * SQ_BLK:(n + 1) * SQ_BLK],
                    start=(k == 0), stop=(k == KT - 1))
            if use_scalar:
                nc.scalar.add(
                    dsts[m][:, n * SQ_BLK:(n + 1) * SQ_BLK],
                    ps[:, 0:SQ_BLK], bias_sb[:, m:m + 1])
            else:
                nc.vector.tensor_scalar_add(
                    dsts[m][:, n * SQ_BLK:(n + 1) * SQ_BLK],
                    ps[:, 0:SQ_BLK], bias_sb[:, m:m + 1])

        # upfront K-proj covers only token-half 0 (sq blocks 0,1) so the
        # first scores wait on 2MB of xk + 1MB of xq instead of all 4MB;
        # token-half 1 (pass B) and q-proj(m1) are deferred into block 0
        kpsA = psc.tile([P, 2 * SQ_BLK], F32, tag="sc", name="kpsA")
        kps1t = [ptr.tile([P, SQ_BLK], F32, tag="tr", name=f"kpA1_{n}")
                 for n in range(2)]
        for k in range(KT):
            for n in range(2):
                nc.tensor.matmul(
                    kpsA[:, n * SQ_BLK:(n + 1) * SQ_BLK],
                    wk_sb[:, k, 0:P],
                    xk_t[k][:, n * SQ_BLK:(n + 1) * SQ_BLK],
                    start=(k == 0), stop=(k == KT - 1))
            for n in range(2):
                nc.tensor.matmul(
                    kps1t[n][:, 0:SQ_BLK],
                    wk_sb[:, k, P:2 * P],
                    xk_t[k][:, n * SQ_BLK:(n + 1) * SQ_BLK],
                    start=(k == 0), stop=(k == KT - 1))
        for n in range(2):
            nc.scalar.add(kT[0][:, n * SQ_BLK:(n + 1) * SQ_BLK],
                          kpsA[:, n * SQ_BLK:(n + 1) * SQ_BLK],
                          bk_sb[:, 0:1])
        for n in range(2):
            nc.vector.tensor_scalar_add(
                kT[1][:, n * SQ_BLK:(n + 1) * SQ_BLK],
                kps1t[n][:, 0:SQ_BLK], bk_sb[:, 1:2])
        qk_proj_blk(xq_t, wq_sb, bq_sb, qT, 0, 0, psc)

        def kproj_passB(m):
            ts = [ptr.tile([P, SQ_BLK], F32, tag="tr", name=f"kpB{m}_{n}")
                  for n in range(2)]
            for k in range(KT):
                for n in range(2):
                    nc.tensor.matmul(
                        ts[n][:, 0:SQ_BLK],
                        wk_sb[:, k, m * P:(m + 1) * P],
                        xk_t[k][:, (2 + n) * SQ_BLK:(3 + n) * SQ_BLK],
                        start=(k == 0), stop=(k == KT - 1))
            for n in range(2):
                nc.vector.tensor_scalar_add(
                    kT[m][:, (2 + n) * SQ_BLK:(3 + n) * SQ_BLK],
                    ts[n][:, 0:SQ_BLK], bk_sb[:, m:m + 1])

        # ---- V projection: v[t, o] = (xT[k,t]).T @ WvT[k, o] (+ bv) ----
        # deferred into attention blocks 0-1 (xv DMA lands mid-block 0)
        def v_proj(tt):
            ps = ptr.tile([P, SQ_BLK], F32, tag="tr", name=f"vps{tt}")
            for k in range(KT):
                nc.tensor.matmul(
                    ps[:, 0:LOCAL_E],
                    xv_t[k][:, tt * P:(tt + 1) * P],
                    wv_sb[:, k, :],
                    start=(k == 0), stop=(k == KT - 1))
            nc.vector.tensor_tensor(
                vbuf.rearrange("p s (h c) -> p s h c", c=VW)[:, tt, :, 0:HD],
                ps[:, 0:LOCAL_E].rearrange("p (h c) -> p h c", c=HD),
                bvb_bc.rearrange("p (h c) -> p h c", c=VW)[:, :, 0:HD],
                mybir.AluOpType.add)

        # ---- software-pipelined attention -----------------------------
        # sq-major block order: output projection of sq can run (and its
        # DMA drain) two blocks after (pr1, sq) completes
        BLKS = [(m, sq) for sq in range(NSQ) for m in range(NPAIR)]
        # deferred q-projection blocks: (bi, sk) -> (n, m); deadline for
        # qT[m][:, n] is block 2n+m (sq-major)
        QPROJ_SCHED = {(1, 10): (1, 0), (1, 15): (1, 1),
                       (2, 2): (2, 0), (2, 15): (2, 1),
                       (4, 2): (3, 0), (4, 10): (3, 1)}

        exs = {}       # b -> list of 16 ex tiles
        po_t = {}      # b -> [po0, po1]
        outU_t = {}    # b -> [128, SQ_BLK] unnormalized (pair-packed rows)
        dn_t = {}      # b -> [2, SQ_BLK] denominators

        FE_A = 128.0 * 1.4426950408889634          # 2^7 * log2(e)
        FE_B = 127.0 * 128.0 - 128.0 * 0.0573 + 0.5  # bias + err-center + rnd

        def em_scores_exp(b, sk, dve=False):
            m, sq = BLKS[b]
            sqs = slice(sq * SQ_BLK, (sq + 1) * SQ_BLK)
            sks = slice(sk * P, (sk + 1) * P)
            ps = psc.tile([P, 2 * SQ_BLK], F32, tag="sc",
                          name=f"sc{b}_{sk}")
            # two K=64 head matmuls packed as PE row-tiles (0,0) and (64,0)
            nc.tensor.matmul(ps[:, 0:SQ_BLK],
                             kT[m][0:HD, sks], qT[m][0:HD, sqs],
                             start=True, stop=True)
            nc.tensor.matmul(ps[:, SQ_BLK:2 * SQ_BLK],
                             kT[m][HD:P, sks], qT[m][HD:P, sqs],
                             start=True, stop=True)
            ex = epool.tile([P, 2 * SQ_BLK], BF16, tag="e",
                            name=f"ex{b}_{sk}")
            if dve:
                # fast-exp bit trick on the (otherwise idle) vector engine:
                # int16(s*FE_A + FE_B) bitcast as bf16 ~ exp(s), +-4.3%
                nc.vector.tensor_scalar(
                    ex[:].bitcast(mybir.dt.int16), ps[:], FE_A, FE_B,
                    mybir.AluOpType.mult, mybir.AluOpType.add)
            else:
                nc.scalar.activation(ex[:], ps[:], EXPF)
            exs[b].append(ex)

        def em_pv(b, sk):
            m, sq = BLKS[b]
            ex = exs[b][sk]
            for X in range(2):
                h = 2 * m + X
                nc.tensor.matmul(
                    po_t[b][X][0:VW, :],
                    vbuf[:, sk, h * VW:(h + 1) * VW],
                    ex[:, X * SQ_BLK:(X + 1) * SQ_BLK],
                    start=(sk == 0), stop=(sk == NSK - 1))

        def em_evict_po(b, drain=False):
            # right after PV(b, 15): free the two po banks quickly; in the
            # drain the scalar engine is idle and takes the outU copies
            oU = spool.tile([P, SQ_BLK], BF16, tag="oU", bufs=2,
                            name=f"oU{b}")
            dn = [spool.tile([1, SQ_BLK], F32, tag="dn", bufs=2,
                             name=f"dn{b}_{X}") for X in range(2)]
            for X in range(2):
                if drain:
                    nc.scalar.copy(oU[X * HD:(X + 1) * HD, :],
                                   po_t[b][X][0:HD, :])
                else:
                    nc.vector.tensor_copy(oU[X * HD:(X + 1) * HD, :],
                                          po_t[b][X][0:HD, :])
                nc.vector.tensor_copy(dn[X][:], po_t[b][X][HD:VW, :])
            outU_t[b] = oU
            dn_t[b] = dn

        bc_t = {}

        def em_normalize_x(b, X, drain=False):
            if X == 0:
                bc_t[b] = spool.tile([P, SQ_BLK], BF16, tag="bc", bufs=2,
                                     name=f"bc{b}")
            rc = spool.tile([1, SQ_BLK], F32, tag="rc", bufs=2,
                            name=f"rc{b}_{X}")
            nc.vector.reciprocal_approx_fast(rc[:], dn_t[b][X][:])
            rcr = spool.tile([1, SQ_BLK], F32R, tag="rcr", bufs=2,
                             name=f"rcr{b}_{X}")
            nc.vector.tensor_copy(rcr[:], rc[:])
            pb = ptr.tile([P, SQ_BLK], F32, tag="tr",
                          name=f"pb{b}_{X}")
            nc.tensor.matmul(pb[0:HD, :], onesr_sb[0:1, 0:HD], rcr[:],
                             start=True, stop=True)
            if drain:
                nc.scalar.copy(bc_t[b][X * HD:(X + 1) * HD, :], pb[0:HD, :])
            else:
                nc.vector.tensor_copy(bc_t[b][X * HD:(X + 1) * HD, :],
                                      pb[0:HD, :])

        def em_normalize_fin(b):
            m, sq = BLKS[b]
            sqs = slice(sq * SQ_BLK, (sq + 1) * SQ_BLK)
            nc.vector.tensor_tensor(outT[m][:, sqs], outU_t[b][:],
                                    bc_t[b][:], mybir.AluOpType.mult)

        fd_t = {}

        def em_oproj(sq, m_range, drain=False):
            sqs = slice(sq * SQ_BLK, (sq + 1) * SQ_BLK)
            if m_range[0] == 0:
                # batch output into 2 wide DMAs per sq (8 chunk-descriptors
                # pay ~1.2us completion each on the queue)
                fd_t[sq] = [fpool.tile([P, 4, SQ_BLK], BF16, tag="fd",
                                       bufs=2, name=f"fd{sq}_{g}")
                            for g in range(2)]
            fd = fd_t[sq]
            ov = outp.rearrange("(g p) t -> p g t", p=P)
            for mm in m_range:
                # in the drain, spread across 4 PSUM slots and both the DVE
                # and the (now idle) scalar engine / scalar DMA queue
                if drain and mm % 2 == 1:
                    pt = ppo.tile([P, SQ_BLK], F32, tag="po",
                                  name=f"op{sq}_{mm}")
                else:
                    pt = ptr.tile([P, SQ_BLK], F32, tag="tr",
                                  name=f"op{sq}_{mm}")
                for kb in range(NPAIR):
                    nc.tensor.matmul(
                        pt[:],
                        wo_sb[:, kb, mm * P:(mm + 1) * P],
                        outT[kb][:, sqs],
                        start=(kb == 0), stop=(kb == NPAIR - 1))
                dst = fd[mm // 4][:, mm % 4, :]
                if drain and mm % 2 == 1:
                    nc.scalar.copy(dst, pt[:])
                else:
                    nc.vector.tensor_copy(dst, pt[:])
                if mm == 3:
                    nc.sync.dma_start(ov[:, 0:4, sqs], fd[0][:])
                if mm == 7:
                    if drain:
                        nc.scalar.dma_start(ov[:, 4:8, sqs], fd[1][:])
                    else:
                        nc.sync.dma_start(ov[:, 4:8, sqs], fd[1][:])

        for bi in range(NBLK + 1):
            drain = bi == NBLK
            if not drain:
                exs[bi] = []
                po_t[bi] = [ppo.tile([VW, SQ_BLK], F32, tag="po",
                                     name=f"po{bi}_{X}") for X in range(2)]
            for sk in range(NSK):
                if bi >= 1:
                    em_pv(bi - 1, sk)
                if not drain:
                    # note: offloading some tiles' exp to the DVE via the
                    # fast-exp bit trick (dve=True) is numerically fine
                    # (rel err ~9e-3) but measured net-neutral-to-worse:
                    # the DVE's bursty FIFO delivers the tiles late and the
                    # psc WAR then stalls the PE score stream.
                    em_scores_exp(bi, sk)
                # deferred startup work in block 0's slack: k-proj token
                # half 1, q-proj (sq0, m1), then the v-projection
                if bi == 0 and sk == 0:
                    kproj_passB(0)
                if bi == 0 and sk == 2:
                    qk_proj_blk(xq_t, wq_sb, bq_sb, qT, 0, 1, psc)
                if bi == 0 and sk == 4:
                    kproj_passB(1)
                if bi == 0 and sk >= 9:
                    v_proj(sk - 9)
                if bi == 1 and sk < 9:
                    v_proj(sk + 7)
                if (bi, sk) in QPROJ_SCHED:
                    n, m = QPROJ_SCHED[(bi, sk)]
                    qk_proj_blk(xq_t, wq_sb, bq_sb, qT, n, m, ptr)
                if bi >= 1 and sk == NSK - 1:
                    em_evict_po(bi - 1, drain=drain)
                if bi >= 2:
                    # stagger block bi-2's post-processing into this block
                    nb = bi - 2
                    if sk == 4:
                        em_normalize_x(nb, 0)
                        em_normalize_x(nb, 1)
                        em_normalize_fin(nb)
                    # o-proj of sq once both its pair-blocks are normalized
                    if bi >= 3 and bi % 2 == 1:
                        osq = (bi - 3) // 2
                        if sk in (8, 10, 12, 14):
                            i0 = (sk - 8) // 2 * 2
                            em_oproj(osq, range(i0, i0 + 2))
        # tail: the last block's normalize + o-proj for the last sq
        em_normalize_x(NBLK - 1, 0)
        em_normalize_x(NBLK - 1, 1)
        em_normalize_fin(NBLK - 1)
        em_oproj(NSQ - 1, range(0, 8), drain=True)

    nc.compile()
    return nc


_NC = None


def _get_nc():
    global _NC
    if _NC is None:
        _NC = _build_program()
    return _NC


def _bf(a):
    return np.ascontiguousarray(a.astype(NPBF16))


def _wre(w):
    """[K*P, N] weight -> [P, K*N] partition-major for contiguous DMA."""
    kp, n = w.shape
    k = kp // P
    return np.ascontiguousarray(
        w.reshape(k, P, n).transpose(1, 0, 2).reshape(P, k * n)
        .astype(NPBF16))


def _make_in_maps(inputs):
    q = np.asarray(inputs["query"], np.float32)
    k = np.asarray(inputs["key"], np.float32)
    v = np.asarray(inputs["value"], np.float32)
    Wq = np.asarray(inputs["Wq"], np.float32)
    Wk = np.asarray(inputs["Wk"], np.float32)
    Wv = np.asarray(inputs["Wv"], np.float32)
    Wo = np.asarray(inputs["Wo"], np.float32)
    bq = np.asarray(inputs["bq"], np.float32)
    bk = np.asarray(inputs["bk"], np.float32)
    bv = np.asarray(inputs["bv"], np.float32)
    scale = np.float32(HD ** -0.5)

    in_maps = []
    for c in range(NCORES):
        b = c // CORES_PER_BATCH
        h0 = (c % CORES_PER_BATCH) * HEADS_PER_CORE
        hsl = slice(h0 * HD, (h0 + HEADS_PER_CORE) * HD)
        bvh = bv[hsl].reshape(HEADS_PER_CORE, HD)
        bvb = np.concatenate(
            [bvh, np.ones((HEADS_PER_CORE, 1), np.float32)], axis=1).ravel()
        in_maps.append({
            "xq": _bf(q[:, b, :].T),
            "xk": _bf(k[:, b, :].T),
            "xv": _bf(v[:, b, :].T),
            "wqt": _wre((Wq[hsl, :] * scale).T),
            "wkt": _wre(Wk[hsl, :].T),
            "wvt": _wre(Wv[hsl, :].T),
            "wot": _wre(Wo[:, hsl].T),
            "bqh": np.ascontiguousarray(bq[hsl] * scale),
            "bkh": np.ascontiguousarray(bk[hsl]),
            "bvb": np.ascontiguousarray(bvb.astype(np.float32)),
            "onescr": np.ones(P, np.float32),
        })
    return in_maps


def run_sharded(inputs, trace=False):
    """Returns (full_output [S,B,E] f32, BassKernelResults)."""
    nc = _get_nc()
    in_maps = _make_in_maps(inputs)
    res = run_bass_kernel_spmd(nc, in_maps, core_ids=list(range(NCORES)),
                               trace=trace)
    bo = np.asarray(inputs["bo"], np.float32)
    final = np.zeros((S, B, E), np.float32)
    for c in range(NCORES):
        b = c // CORES_PER_BATCH
        final[:, b, :] += res.results[c]["outp"].T.astype(np.float32)
    final += bo
    return final, res


def kernel(**inputs):
    out, _ = run_sharded(inputs, trace=False)
    return out
